# revision 4
# baseline (speedup 1.0000x reference)
"""Braid causal self-attention (sigmoid attention + RoPE + QK RMS-norm) on 8
Trainium2 NeuronCores, tensor-parallel over heads (2 heads per core).

v2 rewrite of the baseline, targeting the PE bottleneck seen in the HW trace
(TensorMatrix 80% busy / 465us):
  - fused QKV: one [128,384] matmul per (tile, c-chunk): 256 matmuls instead
    of 768, and w streams while the xT chunk is stationary.
  - ALL transposes moved off the PE onto the DMA transpose xbar (x tiles and
    the per-tile q/k transposes): PE runs matmuls only.
  - attn@v keeps v stationary and streams attn: yT comes out directly in
    [d, q] layout (no y transposes).
  - sigmoid in 1024-wide pairs straight out of 2-bank PSUM tiles with the
    1/sqrt(D) score scale folded into the activation scale.
  - causal masking via a precomputed 4-block mask (one affine_select) and one
    DVE multiply per diagonal pair.
  - per-qc output projection fused into phase 2 (PE slack under the
    ACT-bound sigmoid stream), bf16 partial outputs DMA'd per 128-row tile.
  - one-pair lookahead emission so PE never waits on the sigmoid.

Sharding identical to baseline: core c owns heads 2c,2c+1; host sums the 8
full-shape bf16 partials (wproj column shard, 1/sqrt(T) folded in).
"""

import sys

sys.path.insert(0, "/opt/trn_rl_repo")

import numpy as np

import concourse.bass as bass
import concourse.mybir as mybir
from concourse import bacc
from concourse.tile import TileContext
from concourse.bass_utils import run_bass_kernel_spmd

T = 4096
C = 1024
N_CORES = 8
D = 64
H_PER_CORE = 2
DSH = D * H_PER_CORE  # 128 per-core qkv width
TT = T // 128  # 32 row tiles
C8 = C // 128  # 8 contraction chunks
QC = T // 512  # 8 q chunks
EPS = 1e-6

F32 = mybir.dt.float32
BF16 = mybir.dt.bfloat16

_COMPILED = None


def _build():
    nc = bacc.Bacc("TRN2", target_bir_lowering=False, debug=False,
                   num_devices=N_CORES, num_swdge_queues=4)

    x_d = nc.dram_tensor("x", [T, C], F32, kind="ExternalInput")
    cos_d = nc.dram_tensor("cos", [T, 32], F32, kind="ExternalInput")
    sin_d = nc.dram_tensor("sin", [T, 32], F32, kind="ExternalInput")
    wqkv_d = nc.dram_tensor("wqkvT", [C, 3 * DSH], F32, kind="ExternalInput")
    wpT_d = nc.dram_tensor("wpT", [DSH, C], F32, kind="ExternalInput")
    out_d = nc.dram_tensor("out", [T, C], BF16, kind="ExternalOutput")

    mul = mybir.AluOpType.mult
    sig = mybir.ActivationFunctionType.Sigmoid

    with TileContext(nc) as tc:
        with (
            tc.tile_pool(name="const", bufs=1) as constp,
            tc.tile_pool(name="res", bufs=1) as resp,
        ):
            # weights / trig / mask
            wqkv_b = constp.tile([128, C8, 3 * DSH], BF16)
            nc.gpsimd.dma_start(
                out=wqkv_b[:, :, :],
                in_=wqkv_d.rearrange("(n p) d -> p n d", p=128))
            wp_b = constp.tile([128, C], BF16)
            nc.gpsimd.dma_start(out=wp_b[:, :], in_=wpT_d[:, :])
            cosr = constp.tile([128, TT, 32], BF16)
            sinr = constp.tile([128, TT, 32], BF16)
            nc.gpsimd.dma_start(
                out=cosr[:, :, :], in_=cos_d.rearrange("(n p) d -> p n d", p=128))
            nc.gpsimd.dma_start(
                out=sinr[:, :, :], in_=sin_d.rearrange("(n p) d -> p n d", p=128))
            # dmask[k, j, q] = 1 where q >= 128j + k (valid causal position for
            # diagonal-region block j), else 0.
            dmask = constp.tile([128, 4, 512], BF16)
            nc.gpsimd.memset(dmask[:, :, :], 0.0)
            nc.gpsimd.affine_select(
                out=dmask[:, :, :], in_=dmask[:, :, :],
                compare_op=mybir.AluOpType.is_gt, fill=1.0, base=0,
                pattern=[[128, 4], [-1, 512]], channel_multiplier=1)

            # residents: qkT[d2, 0=q/1=k, ti, t_local], vN[t_local, ti, d2]
            qkT = resp.tile([128, 2, TT, 128], BF16)
            vN = resp.tile([128, TT, DSH], BF16)

            # ---------------- phase 1: QKV + RoPE + RMS ----------------
            with (
                tc.tile_pool(name="p1x", bufs=3) as p1x,
                tc.tile_pool(name="p1w", bufs=2) as p1w,
                tc.tile_pool(name="p1ps", bufs=2, space="PSUM") as p1ps,
            ):
                for ti in range(TT):
                    r0 = ti * 128
                    xt = p1x.tile([128, C], BF16, tag="xt")
                    nc.gpsimd.dma_start(out=xt[:, :], in_=x_d[r0:r0 + 128, :])
                    # xbar transpose: xT[p, m, t] = xt[t, 128m + p]
                    xT = p1x.tile([128, C8, 128], BF16, tag="xT")
                    nc.sync.dma_start_transpose(xT[:, :, :], xt[:, :])

                    qkv_ps = p1ps.tile([128, 3 * DSH], F32, tag="qkv")
                    for c8 in range(C8):
                        nc.tensor.matmul(qkv_ps[:, :], xT[:, c8, :],
                                         wqkv_b[:, c8, :],
                                         start=(c8 == 0), stop=(c8 == C8 - 1))

                    # evacuate q|k (DVE) and v (ACT)
                    qk = p1w.tile([128, 256], BF16, tag="qk")
                    nc.vector.tensor_copy(qk[:, :], qkv_ps[:, 0:256])
                    nc.scalar.copy(vN[:, ti, :], qkv_ps[:, 256:384])

                    # RoPE for q & k fused: layout (proj2, h2, half2, x32)
                    qk4 = qk[:, :].rearrange(
                        "p (pr h f x) -> p pr h f x", pr=2, h=2, f=2)
                    x1 = qk4[:, :, :, 0, :]
                    x2 = qk4[:, :, :, 1, :]
                    c_b = cosr[:, ti, :].unsqueeze(1).unsqueeze(1).broadcast_to((128, 2, 2, 32))
                    s_b = sinr[:, ti, :].unsqueeze(1).unsqueeze(1).broadcast_to((128, 2, 2, 32))
                    rot = p1w.tile([128, 256], BF16, tag="rot")
                    rot4 = rot[:, :].rearrange(
                        "p (pr h f x) -> p pr h f x", pr=2, h=2, f=2)
                    r1 = rot4[:, :, :, 0, :]
                    r2 = rot4[:, :, :, 1, :]
                    tmp = p1w.tile([128, 2, 2, 32], BF16, tag="tmp")
                    tmp2 = p1w.tile([128, 2, 2, 32], BF16, tag="tmp2")
                    # r1 = x1*c + x2*s ; r2 = x2*c - x1*s
                    nc.vector.tensor_tensor(tmp[:, :, :, :], x2, s_b, mul)
                    nc.vector.tensor_tensor(r1, x1, c_b, mul)
                    nc.vector.tensor_add(r1, r1, tmp[:, :, :, :])
                    nc.vector.tensor_tensor(tmp2[:, :, :, :], x1, s_b, mul)
                    nc.vector.tensor_tensor(r2, x2, c_b, mul)
                    nc.vector.tensor_sub(r2, r2, tmp2[:, :, :, :])

                    # RMS-norm: inv = 1/sqrt(ssq/64 + eps), applied to q and k
                    # (1/sqrt(D) score scale folded into sigmoid scale below)
                    sq = p1w.tile([128, 256], BF16, tag="sq")
                    nc.gpsimd.tensor_mul(sq[:, :], rot[:, :], rot[:, :])
                    ssq = p1w.tile([128, 4], F32, tag="ssq")
                    nc.vector.reduce_sum(
                        ssq[:, :], sq[:, :].rearrange("p (g x) -> p g x", g=4),
                        axis=mybir.AxisListType.X)
                    nc.vector.tensor_scalar_add(ssq[:, :], ssq[:, :],
                                                64.0 * EPS)
                    rec = p1w.tile([128, 4], F32, tag="rec")
                    nc.vector.reciprocal(rec[:, :], ssq[:, :])
                    inv = p1w.tile([128, 4], BF16, tag="inv")
                    nc.scalar.activation(inv[:, :], rec[:, :],
                                         mybir.ActivationFunctionType.Sqrt,
                                         bias=0.0, scale=64.0)
                    nrm = p1w.tile([128, 256], BF16, tag="nrm")
                    inv_b = inv[:, :].unsqueeze(2).broadcast_to((128, 4, 64))
                    nc.vector.tensor_tensor(
                        nrm[:, :].rearrange("p (g x) -> p g x", g=4),
                        rot[:, :].rearrange("p (g x) -> p g x", g=4),
                        inv_b, mul)

                    # q/k transposes via xbar into residents
                    nc.sync.dma_start_transpose(qkT[:, 0, ti, :],
                                                nrm[:, 0:128])
                    nc.sync.dma_start_transpose(qkT[:, 1, ti, :],
                                                nrm[:, 128:256])

            # ------------- phase 2+3: attention + projection -------------
            with (
                tc.tile_pool(name="p2at", bufs=3) as p2a,
                tc.tile_pool(name="p2s", bufs=2, space="PSUM") as p2s,
                tc.tile_pool(name="p2y", bufs=2, space="PSUM") as p2y,
                tc.tile_pool(name="p2o", bufs=2, space="PSUM") as p2o,
            ):
                def emit_proj(qc, yT_ps):
                    yT_sb = p2a.tile([128, 512], BF16, tag="yT")
                    nc.vector.tensor_copy(yT_sb[:, :], yT_ps[:, :])
                    for qs in range(4):
                        o_sb = p2a.tile([128, C], BF16, tag="osb")
                        for oc in range(2):
                            o_ps = p2o.tile([128, 512], F32, tag="o")
                            nc.tensor.matmul(
                                o_ps[:, :],
                                yT_sb[:, qs * 128:(qs + 1) * 128],
                                wp_b[:, oc * 512:(oc + 1) * 512],
                                start=True, stop=True)
                            nc.vector.tensor_copy(
                                o_sb[:, oc * 512:(oc + 1) * 512], o_ps[:, :])
                        r0 = qc * 512 + qs * 128
                        nc.gpsimd.dma_start(out=out_d[r0:r0 + 128, :],
                                            in_=o_sb[:, :])

                # flat list of (qc, h, pi) units with one-pair lookahead
                units = []
                for qc in range(QC):
                    for h in range(H_PER_CORE):
                        for pi in range(2 * qc + 2):
                            units.append((qc, h, pi))

                yT_by_qc = {}
                pend = None  # (unit, at_tile)

                def emit_av(unit, at):
                    qc, h, pi = unit
                    nkt = 4 * qc + 4
                    hs = h * 64
                    yT_ps = yT_by_qc[qc]
                    for j in range(2):
                        kt = 2 * pi + j
                        nc.tensor.matmul(
                            yT_ps[hs:hs + 64, :], vN[:, kt, hs:hs + 64],
                            at[:, j, :],
                            start=(kt == 0), stop=(kt == nkt - 1))
                    if h == 1 and pi == 2 * qc + 1:
                        emit_proj(qc, yT_by_qc.pop(qc))

                for unit in units:
                    qc, h, pi = unit
                    npair = 2 * qc + 2
                    hs = h * 64
                    if qc not in yT_by_qc:
                        yT_by_qc[qc] = p2y.tile([128, 512], F32, tag="y", name="yT_ps")
                    s_ps = p2s.tile([128, 2, 512], F32, tag="s")
                    q_rhs = qkT[hs:hs + 64, 0, 4 * qc:4 * qc + 4, :]
                    for j in range(2):
                        kt = 2 * pi + j
                        nc.tensor.matmul(s_ps[:, j, :],
                                         qkT[hs:hs + 64, 1, kt, :], q_rhs,
                                         start=True, stop=True)
                    at = p2a.tile([128, 2, 512], BF16, tag="at")
                    nc.scalar.activation(at[:, :, :], s_ps[:, :, :], sig,
                                         scale=0.125)
                    if pi >= npair - 2:
                        jd = 2 * (pi - (npair - 2))
                        nc.vector.tensor_tensor(at[:, :, :], at[:, :, :],
                                                dmask[:, jd:jd + 2, :], mul)
                    if pend is not None:
                        emit_av(*pend)
                    pend = (unit, at)
                if pend is not None:
                    emit_av(*pend)

    nc.compile()
    return nc


def prepare_in_maps(x, cos, sin, wq, wk, wv, wproj):
    x2d = np.ascontiguousarray(x.reshape(T, C), dtype=np.float32)
    cos = np.ascontiguousarray(cos, dtype=np.float32)
    sin = np.ascontiguousarray(sin, dtype=np.float32)
    in_maps = []
    for c in range(N_CORES):
        sl = slice(c * DSH, (c + 1) * DSH)
        wqkv = np.concatenate(
            [wq[sl, :].T, wk[sl, :].T, wv[sl, :].T], axis=1)
        in_maps.append({
            "x": x2d,
            "cos": cos,
            "sin": sin,
            "wqkvT": np.ascontiguousarray(wqkv, dtype=np.float32),
            # fold y/(sqrt(T)+1e-6) into the projection weights
            "wpT": np.ascontiguousarray(wproj[:, sl].T)
            * np.float32(1.0 / (64.0 + 1e-6)),
        })
    return in_maps


def gather(results):
    acc = np.zeros((T, C), dtype=np.float32)
    for c in range(N_CORES):
        acc += np.asarray(results[c]["out"], dtype=np.float32)
    return acc.reshape(1, T, C)


def kernel(x, cos, sin, wq, wk, wv, wproj):
    global _COMPILED
    if _COMPILED is None:
        _COMPILED = _build()
    nc = _COMPILED
    in_maps = prepare_in_maps(x, cos, sin, wq, wk, wv, wproj)
    res = run_bass_kernel_spmd(nc, in_maps, list(range(N_CORES)))
    return gather(res.results)
